# revision 6
# baseline (speedup 1.0000x reference)
"""MultiLabelSupConLoss Trainium2 kernel (8-core SPMD, Bass/Tile).

Math
----
reference computes, with l_ij = <f0_i, f0_j>/T (f0 = features[:,0,:]):
    logits_max_i = max_j over the full [2B] row of contrast similarities
    e = exp(l[:B,:B] - logits_max)
    per_row = log(sum_j e_ij) - log(sum_{j in pos(i)} e_ij)
    loss = mean over rows with >=1 positive

per_row is invariant to ANY per-row shift c_i (it cancels in the
log-difference), so instead of the full-row max we use c_i = l_ii
(the self-similarity, which dominates every row by a huge margin for
normalized-random features; using it keeps exp() in range exactly like
the reference's row max does).  This removes the need to ever compute
the second half [B:2B] of the contrast matrix: those columns only
entered through logits_max.

The positive mask sim_ij >= 0.5 with sim = inter/(union+1e-6) is
equivalent (integer label counts) to z_ij = 3*inter - rs_i - rs_j >= 1,
computed by a single augmented matmul over K=102 (padded to 128):
    lhsT rows: [labels.T ; ones ; rs ; 0...],
    rhs rows:  [3*labels.T ; -rs ; -ones ; 0...]

Sharding: data-parallel over rows; each of the 8 cores handles 512 rows
and returns per-row (den, pos) partial sums; the host does the final
log/mean (a 4096-element epilogue).

Per core device pipeline, per (i-chunk 128 rows x 1024-col chunk):
    PE : l  = f0T_blk.T @ f0T       -> PSUM (bf16 in, fp32 acc)
    PE : z  = labAug_blk.T @ labAug -> PSUM
    ACT: e  = exp(l + bias_i), accum_out -> den partial   (1 op per block)
    DVE: (z >= 0.5) * e,      accum_out -> pos partial    (1 fused op)
ACT and DVE are the bottleneck engines (both ~19us at 1 elem/cyc/lane);
uniform 1024-wide blocks minimize their per-op overhead while keeping
both PSUM operand pools double-buffered (8 banks total).

DMA: the two HWDGE rings (sync / scalar) each drain FIFO, so the inputs
are packed host-side into two dram blobs -- [fTb | fT] on the sync ring
and [bias | labL | labR] on the scalar ring -- letting ONE transfer per
ring (~386KB) deliver everything block 0 needs.  The remaining columns
follow in 1024-col transfers in need order while compute runs.
"""

import numpy as np
import ml_dtypes

import concourse.bass as bass
import concourse.bacc as bacc
import concourse.mybir as mybir
from concourse import tile
from concourse.bass_utils import run_bass_kernel_spmd

B = 4096
D = 128
N_CORES = 8
ROWS = B // N_CORES          # 512 rows per core
ICHUNK = 128                 # rows per i-chunk (PSUM partition dim)
IC = ROWS // ICHUNK          # 4
CHUNKS = [512, 1024, 1024, 1024, 512]   # ramp-in / steady / tail-out
CH_OFF = [sum(CHUNKS[:i]) for i in range(len(CHUNKS))]
NCH = len(CHUNKS)
CW = 1024                    # max column-block width (2 PSUM banks)
KLAB = 128                   # 100 label dims + 2 augmentation rows + pad
TEMP = 0.07

FCOLS = ROWS + B             # [fTb | fT] packed width
LCOLS = 8 + ROWS + B         # [bias(4xf32 as 8 cols) | labL | labR]

BF16 = ml_dtypes.bfloat16

_cached = None


def _build_nc():
    f32 = mybir.dt.float32
    bf16 = mybir.dt.bfloat16
    nc = bacc.Bacc(
        "TRN2",
        target_bir_lowering=False,
        debug=False,
        num_devices=N_CORES,
    )

    fp_d = nc.dram_tensor("fpack", [D, FCOLS], bf16, kind="ExternalInput")
    lp_d = nc.dram_tensor("lpack", [KLAB, LCOLS], bf16, kind="ExternalInput")
    den_d = nc.dram_tensor("den", [ICHUNK, IC * NCH], f32, kind="ExternalOutput")
    pos_d = nc.dram_tensor("pos", [ICHUNK, IC * NCH], f32, kind="ExternalOutput")

    act_exp = mybir.ActivationFunctionType.Exp

    with tile.TileContext(nc) as tc:
        with (
            tc.tile_pool(name="const", bufs=1) as cpool,
            tc.tile_pool(name="e", bufs=3) as epool,
            tc.tile_pool(name="em", bufs=2) as empool,
            tc.tile_pool(name="psl", bufs=2, space="PSUM") as psl,
            tc.tile_pool(name="psz", bufs=2, space="PSUM") as psz,
        ):
            fp_s = cpool.tile([D, FCOLS], bf16)
            lp_s = cpool.tile([KLAB, LCOLS], bf16)
            den_s = cpool.tile([ICHUNK, IC * NCH], f32)
            pos_s = cpool.tile([ICHUNK, IC * NCH], f32)
            scratch = cpool.tile([1, 8], f32)

            fTb_s = fp_s[:, 0:ROWS]
            labL_s = lp_s[:, 8 : 8 + ROWS]
            bias_s = lp_s[:, 0:8].bitcast(f32)      # [128, 4] fp32

            def fT(c0, c1):
                return fp_s[:, ROWS + c0 : ROWS + c1]

            def labR(c0, c1):
                return lp_s[:, 8 + ROWS + c0 : 8 + ROWS + c1]

            # Two parallel FIFO rings; transfer #1 on each carries all of
            # block 0's operands, later 1024-col transfers follow in need
            # order behind compute.
            nc.sync.dma_start(
                fp_s[:, 0 : ROWS + CHUNKS[0]], fp_d[:, 0 : ROWS + CHUNKS[0]]
            )
            nc.scalar.dma_start(
                lp_s[:, 0 : 8 + ROWS + CHUNKS[0]],
                lp_d[:, 0 : 8 + ROWS + CHUNKS[0]],
            )
            for ch in range(1, NCH):
                a = ROWS + CH_OFF[ch]
                w = CHUNKS[ch]
                nc.sync.dma_start(fp_s[:, a : a + w], fp_d[:, a : a + w])
                a = 8 + ROWS + CH_OFF[ch]
                nc.scalar.dma_start(lp_s[:, a : a + w], lp_d[:, a : a + w])

            # pre-load the exp spline tables while input DMAs stream
            nc.vector.memset(scratch[:], 0.0)
            nc.scalar.activation(
                scratch[:], scratch[:], act_exp, bias=scratch[:, 0:1]
            )

            # short PE clock warm-up (1.2 -> 2.4 GHz) inside the DMA
            # shadow; the first real matmuls finish the ramp with the
            # ~300ns/block of PE slack vs the ACT/DVE cadence.
            warm = cpool.tile([ICHUNK, 512], bf16)
            nc.vector.memset(warm[:], 0.0)
            wps = psz.tile([ICHUNK, CW], f32, tag="z_ps")
            for _ in range(4):
                nc.tensor.matmul(wps[:, :512], warm[:, :ICHUNK], warm[:])

            # column-block outer, row-chunk inner: only block 0 gates the
            # first matmul; later blocks stream in behind compute.
            for ch in range(NCH):
                w = CHUNKS[ch]
                for ic in range(IC):
                    isl = slice(ic * ICHUNK, (ic + 1) * ICHUNK)
                    col = ch * IC + ic

                    l_ps = psl.tile([ICHUNK, w], f32, tag="l_ps")
                    z_ps = psz.tile([ICHUNK, w], f32, tag="z_ps")
                    for h in range(w // 512):
                        j0 = CH_OFF[ch] + h * 512
                        hsl = slice(h * 512, (h + 1) * 512)
                        nc.tensor.matmul(
                            l_ps[:, hsl], fTb_s[:, isl], fT(j0, j0 + 512)
                        )
                    for h in range(w // 512):
                        j0 = CH_OFF[ch] + h * 512
                        hsl = slice(h * 512, (h + 1) * 512)
                        nc.tensor.matmul(
                            z_ps[:, hsl], labL_s[:, isl], labR(j0, j0 + 512)
                        )

                    e_t = epool.tile([ICHUNK, w], f32, tag="e")
                    nc.scalar.activation(
                        e_t[:],
                        l_ps[:],
                        act_exp,
                        bias=bias_s[:, ic : ic + 1],
                        scale=1.0,
                        accum_out=den_s[:, col : col + 1],
                    )

                    em_t = empool.tile([ICHUNK, w], bf16, tag="em")
                    nc.vector.scalar_tensor_tensor(
                        em_t[:],
                        z_ps[:],
                        0.5,
                        e_t[:],
                        op0=mybir.AluOpType.is_ge,
                        op1=mybir.AluOpType.mult,
                        accum_out=pos_s[:, col : col + 1],
                    )

            # Outputs go out in two pieces per tensor: the ch<last columns
            # as soon as their accumulations finish (their ~2us HBM write
            # receipt then overlaps the remaining compute), and the tiny
            # last-chunk columns at the end.  Host folds chunk partials.
            nc.scalar.dma_start(den_d[:], den_s[:])
            nc.sync.dma_start(pos_d[:], pos_s[:])

    nc.compile()
    names = {"fpack": fp_d.name, "lpack": lp_d.name,
             "den": den_d.name, "pos": pos_d.name}
    return nc, names


def _get_nc():
    global _cached
    if _cached is None:
        _cached = _build_nc()
    return _cached


def _prep_inputs(features, labels):
    """Host-side shard prep: packed/transposed/casted operands per core."""
    f0 = np.asarray(features)[:, 0, :].astype(np.float32)      # [B, D]
    lab = np.asarray(labels).astype(np.float32)                # [B, 100]

    s = np.float32(1.0) / np.float32(np.sqrt(np.float32(TEMP)))
    fT16 = np.ascontiguousarray((f0 * s).T).astype(BF16)       # [D, B] bf16
    # row self-similarity (= diagonal of l), from the same bf16 values
    c = (fT16.astype(np.float32) ** 2).sum(axis=0, dtype=np.float32)  # [B]

    rs = lab.sum(axis=1, dtype=np.float32)                     # [B] integers
    labT = lab.T                                               # [100, B]
    L = np.zeros((KLAB, B), dtype=np.float32)
    L[:100] = labT
    L[100] = 1.0
    L[101] = rs
    R = np.zeros((KLAB, B), dtype=np.float32)
    R[:100] = 3.0 * labT
    R[100] = -rs
    R[101] = -1.0
    L16 = L.astype(BF16)
    R16 = R.astype(BF16)

    nc, names = _get_nc()
    in_maps = []
    for core in range(N_CORES):
        blk = slice(core * ROWS, (core + 1) * ROWS)

        fpack = np.empty((D, FCOLS), dtype=BF16)
        fpack[:, :ROWS] = fT16[:, blk]
        fpack[:, ROWS:] = fT16

        bias = np.ascontiguousarray(
            (-c[blk]).reshape(IC, ICHUNK).T.astype(np.float32)
        )  # [128, IC]
        lpack = np.empty((KLAB, LCOLS), dtype=BF16)
        lpack[:, 0:8] = bias.view(BF16)
        lpack[:, 8 : 8 + ROWS] = L16[:, blk]
        lpack[:, 8 + ROWS :] = R16

        in_maps.append({names["fpack"]: fpack, names["lpack"]: lpack})
    return nc, names, in_maps


def _finish(results, names):
    """Host epilogue: per-row log-ratio + masked mean over 4096 rows."""
    den = np.empty(B, dtype=np.float32)
    pos = np.empty(B, dtype=np.float32)
    for core, r in enumerate(results):
        blk = slice(core * ROWS, (core + 1) * ROWS)
        # [128, IC*NCH] chunk partials -> [128, IC] row sums -> row order
        dc = r[names["den"]].reshape(ICHUNK, NCH, IC).sum(axis=1, dtype=np.float32)
        pc = r[names["pos"]].reshape(ICHUNK, NCH, IC).sum(axis=1, dtype=np.float32)
        den[blk] = dc.T.reshape(ROWS)
        pos[blk] = pc.T.reshape(ROWS)
    has = pos > 0
    per_row = np.zeros(B, dtype=np.float32)
    per_row[has] = np.log(den[has]) - np.log(pos[has])
    count = np.float32(max(int(has.sum()), 1))
    loss = np.float32(per_row.sum(dtype=np.float32) / count)
    return np.asarray(loss, dtype=np.float32)


def kernel(features, labels):
    nc, names, in_maps = _prep_inputs(features, labels)
    res = run_bass_kernel_spmd(nc, in_maps, list(range(N_CORES)))
    return _finish(res.results, names)


def kernel_with_results(features, labels, **spmd_kwargs):
    """Like kernel() but also returns the BassKernelResults (for tracing)."""
    nc, names, in_maps = _prep_inputs(features, labels)
    res = run_bass_kernel_spmd(nc, in_maps, list(range(N_CORES)), **spmd_kwargs)
    return _finish(res.results, names), res


# revision 7
# speedup vs baseline: 1.1209x; 1.1209x over previous
"""MultiLabelSupConLoss Trainium2 kernel (8-core SPMD, Bass/Tile).

Math
----
reference computes, with l_ij = <f0_i, f0_j>/T (f0 = features[:,0,:]):
    logits_max_i = max_j over the full [2B] row of contrast similarities
    e = exp(l[:B,:B] - logits_max)
    per_row = log(sum_j e_ij) - log(sum_{j in pos(i)} e_ij)
    loss = mean over rows with >=1 positive

per_row is invariant to ANY per-row shift c_i (it cancels in the
log-difference), so instead of the full-row max we use c_i = l_ii
(the self-similarity, which dominates every row by a huge margin for
normalized-random features; using it keeps exp() in range exactly like
the reference's row max does).  This removes the need to ever compute
the second half [B:2B] of the contrast matrix: those columns only
entered through logits_max.

The positive mask sim_ij >= 0.5 with sim = inter/(union+1e-6) is
equivalent (integer label counts) to z_ij = 3*inter - rs_i - rs_j >= 1,
computed by a single augmented matmul over K=102 (padded to 128):
    lhsT rows: [labels.T ; ones ; rs ; 0...],
    rhs rows:  [3*labels.T ; -rs ; -ones ; 0...]

Sharding: data-parallel over rows; each of the 8 cores handles 512 rows
and returns per-row (den, pos) partial sums; the host does the final
log/mean (a 4096-element epilogue).

Per core device pipeline, per (i-chunk 128 rows x 1024-col chunk):
    PE : l  = f0T_blk.T @ f0T       -> PSUM (bf16 in, fp32 acc)
    PE : z  = labAug_blk.T @ labAug -> PSUM
    ACT: e  = exp(l + bias_i), accum_out -> den partial   (1 op per block)
    DVE: (z >= 0.5) * e,      accum_out -> pos partial    (1 fused op)
ACT and DVE are the bottleneck engines (both ~19us at 1 elem/cyc/lane);
uniform 1024-wide blocks minimize their per-op overhead while keeping
both PSUM operand pools double-buffered (8 banks total).

DMA: the two HWDGE rings (sync / scalar) each drain FIFO, so the inputs
are packed host-side into two dram blobs -- [fTb | fT] on the sync ring
and [bias | labL | labR] on the scalar ring -- letting ONE transfer per
ring (~386KB) deliver everything block 0 needs.  The remaining columns
follow in 1024-col transfers in need order while compute runs.
"""

import numpy as np
import ml_dtypes

import concourse.bass as bass
import concourse.bacc as bacc
import concourse.mybir as mybir
from concourse import tile
from concourse.bass_utils import run_bass_kernel_spmd

B = 4096
D = 128
N_CORES = 8
ROWS = B // N_CORES          # 512 rows per core
ICHUNK = 128                 # rows per i-chunk (PSUM partition dim)
IC = ROWS // ICHUNK          # 4
CHUNKS = [512, 1024, 1024, 1024, 512]   # ramp-in / steady / tail-out
CH_OFF = [sum(CHUNKS[:i]) for i in range(len(CHUNKS))]
NCH = len(CHUNKS)
CW = 1024                    # max column-block width (2 PSUM banks)
KLAB = 128                   # 100 label dims + 2 augmentation rows + pad
TEMP = 0.07

FCOLS = ROWS + B             # [fTb | fT] packed width
LCOLS = 8 + ROWS + B         # [bias(4xf32 as 8 cols) | labL | labR]

BF16 = ml_dtypes.bfloat16

_cached = None


def _build_nc():
    f32 = mybir.dt.float32
    bf16 = mybir.dt.bfloat16
    nc = bacc.Bacc(
        "TRN2",
        target_bir_lowering=False,
        debug=False,
        num_devices=N_CORES,
    )

    fp_d = nc.dram_tensor("fpack", [D, FCOLS], bf16, kind="ExternalInput")
    lp_d = nc.dram_tensor("lpack", [KLAB, LCOLS], bf16, kind="ExternalInput")
    den_d = nc.dram_tensor("den", [ICHUNK, IC * NCH], f32, kind="ExternalOutput")
    pos_d = nc.dram_tensor("pos", [ICHUNK, IC * NCH], f32, kind="ExternalOutput")

    act_exp = mybir.ActivationFunctionType.Exp

    with tile.TileContext(nc) as tc:
        with (
            tc.tile_pool(name="const", bufs=1) as cpool,
            tc.tile_pool(name="e", bufs=3) as epool,
            tc.tile_pool(name="em", bufs=2) as empool,
            tc.tile_pool(name="psl", bufs=2, space="PSUM") as psl,
            tc.tile_pool(name="psz", bufs=2, space="PSUM") as psz,
        ):
            fp_s = cpool.tile([D, FCOLS], bf16)
            lp_s = cpool.tile([KLAB, LCOLS], bf16)
            den_s = cpool.tile([ICHUNK, IC * NCH], f32)
            pos_s = cpool.tile([ICHUNK, IC * NCH], f32)
            scratch = cpool.tile([1, 8], f32)

            fTb_s = fp_s[:, 0:ROWS]
            labL_s = lp_s[:, 8 : 8 + ROWS]
            bias_s = lp_s[:, 0:8].bitcast(f32)      # [128, 4] fp32

            def fT(c0, c1):
                return fp_s[:, ROWS + c0 : ROWS + c1]

            def labR(c0, c1):
                return lp_s[:, 8 + ROWS + c0 : 8 + ROWS + c1]

            # Two parallel FIFO rings; transfer #1 on each carries all of
            # block 0's operands, later 1024-col transfers follow in need
            # order behind compute.
            nc.sync.dma_start(
                fp_s[:, 0 : ROWS + CHUNKS[0]], fp_d[:, 0 : ROWS + CHUNKS[0]]
            )
            nc.scalar.dma_start(
                lp_s[:, 0 : 8 + ROWS + CHUNKS[0]],
                lp_d[:, 0 : 8 + ROWS + CHUNKS[0]],
            )
            for ch in range(1, NCH):
                a = ROWS + CH_OFF[ch]
                w = CHUNKS[ch]
                nc.sync.dma_start(fp_s[:, a : a + w], fp_d[:, a : a + w])
                a = 8 + ROWS + CH_OFF[ch]
                nc.scalar.dma_start(lp_s[:, a : a + w], lp_d[:, a : a + w])

            # pre-load the exp spline tables while input DMAs stream
            nc.vector.memset(scratch[:], 0.0)
            nc.scalar.activation(
                scratch[:], scratch[:], act_exp, bias=scratch[:, 0:1]
            )

            # short PE clock warm-up (1.2 -> 2.4 GHz) inside the DMA
            # shadow; the first real matmuls finish the ramp with the
            # ~300ns/block of PE slack vs the ACT/DVE cadence.
            warm = cpool.tile([ICHUNK, 512], bf16)
            nc.vector.memset(warm[:], 0.0)
            wps = psz.tile([ICHUNK, CW], f32, tag="z_ps")
            for _ in range(3):
                nc.tensor.matmul(wps[:, :512], warm[:, :ICHUNK], warm[:])

            # column-block outer, row-chunk inner: only block 0 gates the
            # first matmul; later blocks stream in behind compute.
            for ch in range(NCH):
                w = CHUNKS[ch]
                for ic in range(IC):
                    isl = slice(ic * ICHUNK, (ic + 1) * ICHUNK)
                    col = ch * IC + ic

                    l_ps = psl.tile([ICHUNK, w], f32, tag="l_ps")
                    z_ps = psz.tile([ICHUNK, w], f32, tag="z_ps")
                    for h in range(w // 512):
                        j0 = CH_OFF[ch] + h * 512
                        hsl = slice(h * 512, (h + 1) * 512)
                        nc.tensor.matmul(
                            l_ps[:, hsl], fTb_s[:, isl], fT(j0, j0 + 512)
                        )
                    for h in range(w // 512):
                        j0 = CH_OFF[ch] + h * 512
                        hsl = slice(h * 512, (h + 1) * 512)
                        nc.tensor.matmul(
                            z_ps[:, hsl], labL_s[:, isl], labR(j0, j0 + 512)
                        )

                    e_t = epool.tile([ICHUNK, w], f32, tag="e")
                    nc.scalar.activation(
                        e_t[:],
                        l_ps[:],
                        act_exp,
                        bias=bias_s[:, ic : ic + 1],
                        scale=1.0,
                        accum_out=den_s[:, col : col + 1],
                    )

                    em_t = empool.tile([ICHUNK, w], bf16, tag="em")
                    nc.vector.scalar_tensor_tensor(
                        em_t[:],
                        z_ps[:],
                        0.5,
                        e_t[:],
                        op0=mybir.AluOpType.is_ge,
                        op1=mybir.AluOpType.mult,
                        accum_out=pos_s[:, col : col + 1],
                    )

            # Outputs go out in two pieces per tensor: the ch<last columns
            # as soon as their accumulations finish (their ~2us HBM write
            # receipt then overlaps the remaining compute), and the tiny
            # last-chunk columns at the end.  Host folds chunk partials.
            nc.scalar.dma_start(den_d[:], den_s[:])
            nc.sync.dma_start(pos_d[:], pos_s[:])

    nc.compile()
    names = {"fpack": fp_d.name, "lpack": lp_d.name,
             "den": den_d.name, "pos": pos_d.name}
    return nc, names


def _get_nc():
    global _cached
    if _cached is None:
        _cached = _build_nc()
    return _cached


def _prep_inputs(features, labels):
    """Host-side shard prep: packed/transposed/casted operands per core."""
    f0 = np.asarray(features)[:, 0, :].astype(np.float32)      # [B, D]
    lab = np.asarray(labels).astype(np.float32)                # [B, 100]

    s = np.float32(1.0) / np.float32(np.sqrt(np.float32(TEMP)))
    fT16 = np.ascontiguousarray((f0 * s).T).astype(BF16)       # [D, B] bf16
    # row self-similarity (= diagonal of l), from the same bf16 values
    c = (fT16.astype(np.float32) ** 2).sum(axis=0, dtype=np.float32)  # [B]

    rs = lab.sum(axis=1, dtype=np.float32)                     # [B] integers
    labT = lab.T                                               # [100, B]
    L = np.zeros((KLAB, B), dtype=np.float32)
    L[:100] = labT
    L[100] = 1.0
    L[101] = rs
    R = np.zeros((KLAB, B), dtype=np.float32)
    R[:100] = 3.0 * labT
    R[100] = -rs
    R[101] = -1.0
    L16 = L.astype(BF16)
    R16 = R.astype(BF16)

    nc, names = _get_nc()
    in_maps = []
    for core in range(N_CORES):
        blk = slice(core * ROWS, (core + 1) * ROWS)

        fpack = np.empty((D, FCOLS), dtype=BF16)
        fpack[:, :ROWS] = fT16[:, blk]
        fpack[:, ROWS:] = fT16

        bias = np.ascontiguousarray(
            (-c[blk]).reshape(IC, ICHUNK).T.astype(np.float32)
        )  # [128, IC]
        lpack = np.empty((KLAB, LCOLS), dtype=BF16)
        lpack[:, 0:8] = bias.view(BF16)
        lpack[:, 8 : 8 + ROWS] = L16[:, blk]
        lpack[:, 8 + ROWS :] = R16

        in_maps.append({names["fpack"]: fpack, names["lpack"]: lpack})
    return nc, names, in_maps


def _finish(results, names):
    """Host epilogue: per-row log-ratio + masked mean over 4096 rows."""
    den = np.empty(B, dtype=np.float32)
    pos = np.empty(B, dtype=np.float32)
    for core, r in enumerate(results):
        blk = slice(core * ROWS, (core + 1) * ROWS)
        # [128, IC*NCH] chunk partials -> [128, IC] row sums -> row order
        dc = r[names["den"]].reshape(ICHUNK, NCH, IC).sum(axis=1, dtype=np.float32)
        pc = r[names["pos"]].reshape(ICHUNK, NCH, IC).sum(axis=1, dtype=np.float32)
        den[blk] = dc.T.reshape(ROWS)
        pos[blk] = pc.T.reshape(ROWS)
    has = pos > 0
    per_row = np.zeros(B, dtype=np.float32)
    per_row[has] = np.log(den[has]) - np.log(pos[has])
    count = np.float32(max(int(has.sum()), 1))
    loss = np.float32(per_row.sum(dtype=np.float32) / count)
    return np.asarray(loss, dtype=np.float32)


def kernel(features, labels):
    nc, names, in_maps = _prep_inputs(features, labels)
    res = run_bass_kernel_spmd(nc, in_maps, list(range(N_CORES)))
    return _finish(res.results, names)


def kernel_with_results(features, labels, **spmd_kwargs):
    """Like kernel() but also returns the BassKernelResults (for tracing)."""
    nc, names, in_maps = _prep_inputs(features, labels)
    res = run_bass_kernel_spmd(nc, in_maps, list(range(N_CORES)), **spmd_kwargs)
    return _finish(res.results, names), res


# revision 11
# speedup vs baseline: 1.1778x; 1.0508x over previous
"""MultiLabelSupConLoss Trainium2 kernel (8-core SPMD, Bass/Tile).

Math
----
reference computes, with l_ij = <f0_i, f0_j>/T (f0 = features[:,0,:]):
    logits_max_i = max_j over the full [2B] row of contrast similarities
    e = exp(l[:B,:B] - logits_max)
    per_row = log(sum_j e_ij) - log(sum_{j in pos(i)} e_ij)
    loss = mean over rows with >=1 positive

per_row is invariant to ANY per-row shift c_i (it cancels in the
log-difference), so instead of the full-row max we use c_i = l_ii
(the self-similarity, which dominates every row by a huge margin for
normalized-random features; using it keeps exp() in range exactly like
the reference's row max does).  This removes the need to ever compute
the second half [B:2B] of the contrast matrix: those columns only
entered through logits_max.

The positive mask sim_ij >= 0.5 with sim = inter/(union+1e-6) is
equivalent (integer label counts) to z_ij = 3*inter - rs_i - rs_j >= 1,
computed by a single augmented matmul over K=102 (padded to 128):
    lhsT rows: [labels.T ; ones ; rs ; 0...],
    rhs rows:  [3*labels.T ; -rs ; -ones ; 0...]

Sharding: data-parallel over rows; each of the 8 cores handles 512 rows
and returns per-row (den, pos) partial sums; the host does the final
log/mean (a 4096-element epilogue).

Per core device pipeline, per (i-chunk 128 rows x 1024-col chunk):
    PE : l  = f0T_blk.T @ f0T       -> PSUM (bf16 in, fp32 acc)
    PE : z  = labAug_blk.T @ labAug -> PSUM
    ACT: e  = exp(l + bias_i), accum_out -> den partial   (1 op per block)
    DVE: (z >= 0.5) * e,      accum_out -> pos partial    (1 fused op)
ACT and DVE are the bottleneck engines (both ~19-20us at 1 elem/cyc/
lane; the stt has no fast DVE uops and its z operand is PSUM-bound, so
1x is a hard floor).  Blocks are [512, 1024, 1024, 1024, 512] columns:
the narrow lead block starts the pipeline ~1.3us earlier, the narrow
tail block shortens the final drain, and the wide middle minimizes
per-op overhead, with both PSUM pools double-buffered (8 banks).

DMA: the two HWDGE rings (sync / scalar) each drain FIFO, so the inputs
are packed host-side into two dram blobs -- [fTb | fT] on the sync ring
and [bias | labL | labR] on the scalar ring -- letting ONE ~256KB
transfer per ring deliver everything block 0 needs.  Remaining columns
follow in need order while compute runs.  All descriptor-expansion
instructions issue up front: a dma_start emitted mid-loop stalls its
sequencer (and the scalar sequencer also runs the ACT stream).

Fixed costs measured on this part: ~6us NEFF preamble, ~2us DMA
first-byte latency, ~4.5us output-DMA + teardown + profiler close
(an empty kernel measures 11.6us), so exec times sit ~12us above the
compute span.
"""

import numpy as np
import ml_dtypes

import concourse.bass as bass
import concourse.bacc as bacc
import concourse.mybir as mybir
from concourse import tile
from concourse.bass_utils import run_bass_kernel_spmd

B = 4096
D = 128
N_CORES = 8
ROWS = B // N_CORES          # 512 rows per core
ICHUNK = 128                 # rows per i-chunk (PSUM partition dim)
IC = ROWS // ICHUNK          # 4
CHUNKS = [512, 1024, 1024, 1024, 512]   # ramp-in / steady / tail-out
CH_OFF = [sum(CHUNKS[:i]) for i in range(len(CHUNKS))]
NCH = len(CHUNKS)
CW = 1024                    # max column-block width (2 PSUM banks)
KLAB = 128                   # 100 label dims + 2 augmentation rows + pad
TEMP = 0.07

FCOLS = ROWS + B             # [fTb | fT] packed width
LCOLS = 8 + ROWS + B         # [bias(4xf32 as 8 cols) | labL | labR]

BF16 = ml_dtypes.bfloat16

_cached = None


def _build_nc():
    f32 = mybir.dt.float32
    bf16 = mybir.dt.bfloat16
    nc = bacc.Bacc(
        "TRN2",
        target_bir_lowering=False,
        debug=False,
        num_devices=N_CORES,
    )

    fp_d = nc.dram_tensor("fpack", [D, FCOLS], bf16, kind="ExternalInput")
    lp_d = nc.dram_tensor("lpack", [KLAB, LCOLS], bf16, kind="ExternalInput")
    den_d = nc.dram_tensor("den", [ICHUNK, IC * NCH], f32, kind="ExternalOutput")
    pos_d = nc.dram_tensor("pos", [ICHUNK, IC * NCH], f32, kind="ExternalOutput")

    act_exp = mybir.ActivationFunctionType.Exp

    with tile.TileContext(nc) as tc:
        with (
            tc.tile_pool(name="const", bufs=1) as cpool,
            tc.tile_pool(name="e", bufs=3) as epool,
            tc.tile_pool(name="em", bufs=2) as empool,
            tc.tile_pool(name="psl", bufs=2, space="PSUM") as psl,
            tc.tile_pool(name="psz", bufs=2, space="PSUM") as psz,
        ):
            fp_s = cpool.tile([D, FCOLS], bf16)
            lp_s = cpool.tile([KLAB, LCOLS], bf16)
            den_s = cpool.tile([ICHUNK, IC * NCH], f32)
            pos_s = cpool.tile([ICHUNK, IC * NCH], f32)
            scratch = cpool.tile([1, 8], f32)

            fTb_s = fp_s[:, 0:ROWS]
            labL_s = lp_s[:, 8 : 8 + ROWS]
            bias_s = lp_s[:, 0:8].bitcast(f32)      # [128, 4] fp32

            def fT(c0, c1):
                return fp_s[:, ROWS + c0 : ROWS + c1]

            def labR(c0, c1):
                return lp_s[:, 8 + ROWS + c0 : 8 + ROWS + c1]

            # Two parallel FIFO rings; transfer #1 on each carries all of
            # block 0's operands, later 1024-col transfers follow in need
            # order behind compute.
            nc.sync.dma_start(
                fp_s[:, 0 : ROWS + CHUNKS[0]], fp_d[:, 0 : ROWS + CHUNKS[0]]
            )
            nc.scalar.dma_start(
                lp_s[:, 0 : 8 + ROWS + CHUNKS[0]],
                lp_d[:, 0 : 8 + ROWS + CHUNKS[0]],
            )
            for ch in range(1, NCH):
                a = ROWS + CH_OFF[ch]
                w = CHUNKS[ch]
                nc.sync.dma_start(fp_s[:, a : a + w], fp_d[:, a : a + w])
                a = 8 + ROWS + CH_OFF[ch]
                nc.scalar.dma_start(lp_s[:, a : a + w], lp_d[:, a : a + w])

            # pre-load the exp spline tables while input DMAs stream
            nc.vector.memset(scratch[:], 0.0)
            nc.scalar.activation(
                scratch[:], scratch[:], act_exp, bias=scratch[:, 0:1]
            )

            # short PE clock warm-up (1.2 -> 2.4 GHz) inside the DMA
            # shadow; the first real matmuls finish the ramp with the
            # ~300ns/block of PE slack vs the ACT/DVE cadence.
            warm = cpool.tile([ICHUNK, 512], bf16)
            nc.vector.memset(warm[:], 0.0)
            wps = psz.tile([ICHUNK, CW], f32, tag="z_ps")
            for _ in range(3):
                nc.tensor.matmul(wps[:, :512], warm[:, :ICHUNK], warm[:])

            # column-block outer, row-chunk inner: only block 0 gates the
            # first matmul; later blocks stream in behind compute.
            for ch in range(NCH):
                w = CHUNKS[ch]
                for ic in range(IC):
                    isl = slice(ic * ICHUNK, (ic + 1) * ICHUNK)
                    col = ch * IC + ic

                    l_ps = psl.tile([ICHUNK, w], f32, tag="l_ps")
                    z_ps = psz.tile([ICHUNK, w], f32, tag="z_ps")
                    for h in range(w // 512):
                        j0 = CH_OFF[ch] + h * 512
                        hsl = slice(h * 512, (h + 1) * 512)
                        nc.tensor.matmul(
                            l_ps[:, hsl], fTb_s[:, isl], fT(j0, j0 + 512)
                        )
                    for h in range(w // 512):
                        j0 = CH_OFF[ch] + h * 512
                        hsl = slice(h * 512, (h + 1) * 512)
                        nc.tensor.matmul(
                            z_ps[:, hsl], labL_s[:, isl], labR(j0, j0 + 512)
                        )

                    e_t = epool.tile([ICHUNK, w], f32, tag="e")
                    nc.scalar.activation(
                        e_t[:],
                        l_ps[:],
                        act_exp,
                        bias=bias_s[:, ic : ic + 1],
                        scale=1.0,
                        accum_out=den_s[:, col : col + 1],
                    )

                    em_t = empool.tile([ICHUNK, w], bf16, tag="em")
                    nc.vector.scalar_tensor_tensor(
                        em_t[:],
                        z_ps[:],
                        0.5,
                        e_t[:],
                        op0=mybir.AluOpType.is_ge,
                        op1=mybir.AluOpType.mult,
                        accum_out=pos_s[:, col : col + 1],
                    )


            # Outputs go out in two pieces per tensor: the ch<last columns
            # as soon as their accumulations finish (their ~2us HBM write
            # receipt then overlaps the remaining compute), and the tiny
            # last-chunk columns at the end.  Host folds chunk partials.
            nc.scalar.dma_start(den_d[:], den_s[:])
            nc.sync.dma_start(pos_d[:], pos_s[:])

    nc.compile()
    names = {"fpack": fp_d.name, "lpack": lp_d.name,
             "den": den_d.name, "pos": pos_d.name}
    return nc, names


def _get_nc():
    global _cached
    if _cached is None:
        _cached = _build_nc()
    return _cached


def _prep_inputs(features, labels):
    """Host-side shard prep: packed/transposed/casted operands per core."""
    f0 = np.asarray(features)[:, 0, :].astype(np.float32)      # [B, D]
    lab = np.asarray(labels).astype(np.float32)                # [B, 100]

    s = np.float32(1.0) / np.float32(np.sqrt(np.float32(TEMP)))
    fT16 = np.ascontiguousarray((f0 * s).T).astype(BF16)       # [D, B] bf16
    # row self-similarity (= diagonal of l), from the same bf16 values
    c = (fT16.astype(np.float32) ** 2).sum(axis=0, dtype=np.float32)  # [B]

    rs = lab.sum(axis=1, dtype=np.float32)                     # [B] integers
    labT = lab.T                                               # [100, B]
    L = np.zeros((KLAB, B), dtype=np.float32)
    L[:100] = labT
    L[100] = 1.0
    L[101] = rs
    R = np.zeros((KLAB, B), dtype=np.float32)
    R[:100] = 3.0 * labT
    R[100] = -rs
    R[101] = -1.0
    L16 = L.astype(BF16)
    R16 = R.astype(BF16)

    nc, names = _get_nc()
    in_maps = []
    for core in range(N_CORES):
        blk = slice(core * ROWS, (core + 1) * ROWS)

        fpack = np.empty((D, FCOLS), dtype=BF16)
        fpack[:, :ROWS] = fT16[:, blk]
        fpack[:, ROWS:] = fT16

        bias = np.ascontiguousarray(
            (-c[blk]).reshape(IC, ICHUNK).T.astype(np.float32)
        )  # [128, IC]
        lpack = np.empty((KLAB, LCOLS), dtype=BF16)
        lpack[:, 0:8] = bias.view(BF16)
        lpack[:, 8 : 8 + ROWS] = L16[:, blk]
        lpack[:, 8 + ROWS :] = R16

        in_maps.append({names["fpack"]: fpack, names["lpack"]: lpack})
    return nc, names, in_maps


def _finish(results, names):
    """Host epilogue: per-row log-ratio + masked mean over 4096 rows."""
    den = np.empty(B, dtype=np.float32)
    pos = np.empty(B, dtype=np.float32)
    for core, r in enumerate(results):
        blk = slice(core * ROWS, (core + 1) * ROWS)
        # [128, IC*NCH] chunk partials -> [128, IC] row sums -> row order
        dc = r[names["den"]].reshape(ICHUNK, NCH, IC).sum(axis=1, dtype=np.float32)
        pc = r[names["pos"]].reshape(ICHUNK, NCH, IC).sum(axis=1, dtype=np.float32)
        den[blk] = dc.T.reshape(ROWS)
        pos[blk] = pc.T.reshape(ROWS)
    has = pos > 0
    per_row = np.zeros(B, dtype=np.float32)
    per_row[has] = np.log(den[has]) - np.log(pos[has])
    count = np.float32(max(int(has.sum()), 1))
    loss = np.float32(per_row.sum(dtype=np.float32) / count)
    return np.asarray(loss, dtype=np.float32)


def kernel(features, labels):
    nc, names, in_maps = _prep_inputs(features, labels)
    res = run_bass_kernel_spmd(nc, in_maps, list(range(N_CORES)))
    return _finish(res.results, names)


def kernel_with_results(features, labels, **spmd_kwargs):
    """Like kernel() but also returns the BassKernelResults (for tracing)."""
    nc, names, in_maps = _prep_inputs(features, labels)
    res = run_bass_kernel_spmd(nc, in_maps, list(range(N_CORES)), **spmd_kwargs)
    return _finish(res.results, names), res
